# revision 38
# baseline (speedup 1.0000x reference)
"""Trainium2 Bass kernel for GatedCrossAttention (B=4, N=4096, C=1024, H=16, M=4).

Reference math (dead code removed: the v/gate projections are overwritten
by views of k in the original module, so v = g = k):
    q = query @ Wq.T + bq                    [B,N,C]   -> [B,N,H,hd]
    k = key   @ Wk.T + bk                    [B,N,M,C] -> [B,N,M,H,hd]
    attn = softmax_M(SCALE * einsum('bnhc,bnmhc->bnmh', q, k))
    out  = einsum('bnmh,bnmhc->bnhc', attn, k*k) . reshape(B,N,C)
    out  = out @ Wo.T + bo

Strategy: pure data parallel over the 16384 tokens (8 cores x 2048), no
collectives.  Channel-major on-chip layout (channels on partitions, tokens on
the free axis) so every projection contraction is a natural PE op.

Attention is computed HEAD-MAJOR via scatter-accumulate indicator matmuls:
logits land in one PSUM tile pslt[(m,h), t] (64 partitions) by accumulating
32 cheap matmuls (stationary = tiny indicator [128,64], moving = q*k product).
The softmax m-sum (a cross-partition reduction) is a single indicator matmul,
so there are NO PE transposes and no LDW-heavy stationary loads.  Softmax
weights are broadcast back to channel rows with indicator matmuls ([64,128]
stationary, w moving), then DVE computes sum_m w*kp*kp with an add tree.

The whole thing is software-pipelined at the source level: attention of block
b-1 is emitted interleaved between the projection matmul groups of block b so
the PE never idles long enough for HAM to re-throttle the clock.
"""

import dataclasses
import numpy as np
from contextlib import ExitStack

try:
    import concourse.bass as bass
except ImportError:  # path fallback for bare containers
    import sys

    sys.path.insert(0, "/opt/trn_rl_repo")
    import concourse.bass as bass

import concourse.tile as tile
from concourse import bacc, mybir
from concourse.bass_utils import run_bass_kernel_spmd

# problem constants (hardcoded per the task contract)
B, N, C, H, HD, M = 4, 4096, 1024, 16, 64, 4
SCALE = float(HD) ** -0.5
NCORES = 8
T_TOTAL = B * N
T_CORE = T_TOTAL // NCORES  # 2048
TB = 256                    # tokens per block
NBLK = T_CORE // TB         # 8
NJ = C // 128               # 8 channel chunks
NT = TB // 128              # token subtiles per block

DT = mybir.dt.float16
NPDT = np.float16
F32 = mybir.dt.float32


def _bcast(ap, reps, axis):
    """Insert a 0-stride dim of size `reps` at AP position `axis` (0=partition)."""
    new = list(ap.ap)
    new.insert(axis, [0, reps])
    return dataclasses.replace(ap, ap=new)


def build_nc(t_core=T_CORE, with_bias=False):
    nblk = t_core // TB
    nc = bacc.Bacc("TRN2", target_bir_lowering=False, debug=False)

    qT = nc.declare_dram_parameter("qT", [C, t_core], DT, isOutput=False)
    # blocked m-pair layout: kT[mp, blk, ch, mi, t] = k[2*mp+mi, ch, blk*TB+t]
    kT = nc.declare_dram_parameter("kT", [2, t_core // TB, C, 2, TB], DT,
                                   isOutput=False)
    wqT = nc.declare_dram_parameter("wqT", [C, C], DT, isOutput=False)
    wkT = nc.declare_dram_parameter("wkT", [C, C], DT, isOutput=False)
    woT = nc.declare_dram_parameter("woT", [C, C], DT, isOutput=False)
    indl = nc.declare_dram_parameter("indl", [128, 16 * 32], DT, isOutput=False)
    inds = nc.declare_dram_parameter("inds", [32, 32], DT, isOutput=False)
    indb = nc.declare_dram_parameter("indb", [32, 16 * 128], DT, isOutput=False)
    if with_bias:
        bqp = nc.declare_dram_parameter("bqp", [128, NJ], F32, isOutput=False)
        bkp = nc.declare_dram_parameter("bkp", [128, NJ], F32, isOutput=False)
        bo = nc.declare_dram_parameter("bo", [1, C], DT, isOutput=False)
    out = nc.declare_dram_parameter("out", [t_core, C], F32, isOutput=True)

    # DRAM views: channel dim split into (chunk, partition)
    qT_v = qT.ap().rearrange("(c p) t -> p c t", p=128)
    kT_v = kT.ap().rearrange("mp b (c p) mi t -> p mp b c mi t", p=128)
    wq_v = wqT.ap().rearrange("(c p) j -> p c j", p=128)
    wk_v = wkT.ap().rearrange("(c p) j -> p c j", p=128)
    wo_v = woT.ap().rearrange("(c p) j -> p c j", p=128)

    with tile.TileContext(nc) as tc, ExitStack() as ctx:
        consts = ctx.enter_context(tc.tile_pool(name="consts", bufs=1))
        p_inq = ctx.enter_context(tc.tile_pool(name="inq", bufs=2))
        p_ink = ctx.enter_context(tc.tile_pool(name="ink", bufs=4))
        p_qp = ctx.enter_context(tc.tile_pool(name="qp", bufs=2))
        p_kp = ctx.enter_context(tc.tile_pool(name="kp", bufs=16))
        p_prod = ctx.enter_context(tc.tile_pool(name="prod", bufs=10))
        p_t1 = ctx.enter_context(tc.tile_pool(name="t1", bufs=2))
        p_ct = ctx.enter_context(tc.tile_pool(name="ct", bufs=2))
        p_tmp = ctx.enter_context(tc.tile_pool(name="tmp", bufs=2))
        p_sm = ctx.enter_context(tc.tile_pool(name="sm", bufs=2))
        p_psb = ctx.enter_context(tc.tile_pool(name="psb", bufs=2))
        p_yb = ctx.enter_context(tc.tile_pool(name="yb", bufs=2))
        p_out = ctx.enter_context(tc.tile_pool(name="outs", bufs=3))
        pp = ctx.enter_context(tc.tile_pool(name="pp", bufs=2, space="PSUM"))
        plsr = ctx.enter_context(tc.tile_pool(name="plsr", bufs=1, space="PSUM"))
        pb = ctx.enter_context(tc.tile_pool(name="pb", bufs=2, space="PSUM"))
        po = ctx.enter_context(tc.tile_pool(name="po", bufs=2, space="PSUM"))

        # ---- constants / weights (resident) ----
        wq_sb = consts.tile([128, NJ, C], DT)
        wk_sb = consts.tile([128, NJ, C], DT)
        wo_sb = consts.tile([128, NJ, C], DT)
        indl_sb = consts.tile([128, 16 * 32], DT)
        inds_sb = consts.tile([32, 32], DT)
        indb_sb = consts.tile([32, 16 * 128], DT)
        # weights go on the scalar HWDGE ring (idle at startup) so the block-0
        # input loads on the sync ring run in parallel with them.
        # wq in two halves (1KB DMA lines) so q-proj r=0..3 can start before
        # the whole 2 MB weight load lands; wk/wo whole (2KB lines, needed
        # later)
        for h in range(2):
            nc.scalar.dma_start(out=wq_sb[:, :, h * 512:(h + 1) * 512],
                                in_=wq_v[:, :, h * 512:(h + 1) * 512])
        nc.scalar.dma_start(out=wk_sb, in_=wk_v)
        nc.scalar.dma_start(out=wo_sb, in_=wo_v)
        if with_bias:
            bq_sb = consts.tile([128, NJ], F32)
            bk_sb = consts.tile([128, NJ], F32)
            bo_sb = consts.tile([1, C], DT)
            ones_sb = consts.tile([1, 128], DT)
            nc.vector.memset(ones_sb, 1.0)
            nc.scalar.dma_start(out=bq_sb, in_=bqp.ap())
            nc.scalar.dma_start(out=bk_sb, in_=bkp.ap())
            nc.scalar.dma_start(out=bo_sb, in_=bo.ap())

        st = [dict() for _ in range(nblk)]  # per-block live tiles
        _ind_loaded = []

        def emit_ind_loads():
            # indicator loads go after block-0's q/k loads on the sync ring:
            # they are only needed once block-0 attention starts
            if not _ind_loaded:
                nc.sync.dma_start(out=indl_sb, in_=indl.ap())
                nc.sync.dma_start(out=inds_sb, in_=inds.ap())
                nc.sync.dma_start(out=indb_sb, in_=indb.ap())
                _ind_loaded.append(True)

        def emit_load(blk):
            t0 = blk * TB
            tsl = slice(t0, t0 + TB)
            q_in = p_inq.tile([128, NJ, TB], DT, tag="qin", name="qin")
            nc.sync.dma_start(out=q_in, in_=qT_v[:, :, tsl])
            # m-pair interleaved so one N=512 matmul projects both m's
            k_in = [p_ink.tile([128, NJ, 2, TB], DT, tag="kin", name="kin")
                    for _ in range(2)]
            for mp in range(2):
                nc.sync.dma_start(out=k_in[mp], in_=kT_v[:, mp, blk])
            st[blk]["q_in"] = q_in
            st[blk]["k_in"] = k_in

        def emit_qproj(blk):
            q_in = st[blk]["q_in"]
            qp = p_qp.tile([128, NJ, TB], DT, tag="qp", name="qp")
            for r in range(NJ):
                ps = pp.tile([128, TB], F32, tag="pp", name="pp")
                for c in range(NJ):
                    nc.tensor.matmul(
                        ps,
                        wq_sb[:, c, r * 128:(r + 1) * 128],
                        q_in[:, c, :],
                        start=(c == 0),
                        stop=(c == NJ - 1),
                    )
                if with_bias:
                    nc.scalar.add(qp[:, r, :], ps, bq_sb[:, r:r + 1])
                else:
                    nc.scalar.copy(out=qp[:, r, :], in_=ps)
            st[blk]["qp"] = qp

        def emit_kproj_pair(blk, mp, r):
            # two m-projections accumulate into one PSUM bank (disjoint
            # column halves), evacuated by a single ACT copy
            k_in = st[blk]["k_in"]
            if "kp" not in st[blk]:
                st[blk]["kp"] = [
                    p_kp.tile([128, M, TB], DT, tag="kp", name="kp")
                    for _ in range(NJ)
                ]
            kp = st[blk]["kp"]
            ps = pp.tile([128, 2, TB], F32, tag="pp", name="pp")
            for c in range(NJ):
                nc.tensor.matmul(
                    ps,
                    wk_sb[:, c, r * 128:(r + 1) * 128],
                    k_in[mp][:, c, :, :],
                    start=(c == 0),
                    stop=(c == NJ - 1),
                )
            if with_bias:
                nc.scalar.add(kp[r][:, 2 * mp:2 * mp + 2, :], ps,
                              bk_sb[:, r:r + 1])
            else:
                nc.scalar.copy(out=kp[r][:, 2 * mp:2 * mp + 2, :], in_=ps)

        def emit_prods(blk):
            qp, kp = st[blk]["qp"], st[blk]["kp"]
            prods = []
            for r in range(NJ):
                prod = p_prod.tile([128, M, TB], DT, tag="prod", name="prod")
                nc.vector.tensor_mul(prod, _bcast(qp[:, r, :], M, 1), kp[r])
                prods.append(prod)
            st[blk]["prods"] = prods

        def emit_logits(blk):
            prods = st[blk]["prods"]
            # logits head-major with mi on the free axis: row = 16*mp + h,
            # free = (mi, t) with m = 2*mp + mi.  pslt [:,0] / srep [:,1]
            # share one plsr tile
            ls = plsr.tile([32, 3, TB], F32, tag="plsr", name="plsr")
            for r in range(NJ):
                for mp in range(2):
                    k = 2 * r + mp
                    nc.tensor.matmul(
                        ls[:, 0:2, :],
                        indl_sb[:, k * 32:(k + 1) * 32],
                        prods[r][:, 2 * mp:2 * mp + 2, :],
                        start=(r == 0 and mp == 0),
                        stop=(r == NJ - 1 and mp == 1),
                    )
            # exp emitted here (not with the rest of softmax) so it sits early
            # in the strict-FIFO ACT queue, ahead of the next kp copies
            e = p_sm.tile([32, 2, TB], DT, tag="e", name="e")
            nc.scalar.activation(
                e, ls[:, 0:2, :], func=mybir.ActivationFunctionType.Exp,
                scale=SCALE,
            )
            st[blk]["ls"] = ls
            st[blk]["e"] = e

        def emit_softmax(blk):
            e = st[blk]["e"]
            srep = st[blk]["ls"][:, 2, :]
            # mi-planes summed by PSUM accumulation across two matmuls
            for mi in range(2):
                nc.tensor.matmul(srep, inds_sb, e[:, mi, :],
                                 start=(mi == 0), stop=(mi == 1))
            rcp = p_sm.tile([32, TB], F32, tag="rcp", name="rcp")
            nc.vector.reciprocal_approx_fast(rcp, srep)
            w = p_sm.tile([32, 2, TB], DT, tag="w", name="w")
            nc.vector.tensor_mul(w, e, _bcast(rcp, 2, 1))
            st[blk]["w"] = w

        def emit_attend(blk, r):
            kp, w = st[blk]["kp"], st[blk]["w"]
            if r == 0:
                st[blk]["yb"] = p_yb.tile([128, NJ, TB], DT, tag="yb", name="yb")
            yb = st[blk]["yb"]
            psb2 = [pb.tile([128, 2, TB], F32, tag="pb", name="pb")
                    for _ in range(2)]
            for mp in range(2):
                k = 2 * r + mp
                nc.tensor.matmul(
                    psb2[mp],
                    indb_sb[:, k * 128:(k + 1) * 128],
                    w,
                    start=True,
                    stop=True,
                )
            psb_sb = p_psb.tile([128, M, TB], DT, tag="psb", name="psb")
            nc.scalar.copy(out=psb_sb[:, 0:2, :], in_=psb2[0])
            nc.scalar.copy(out=psb_sb[:, 2:4, :], in_=psb2[1])
            t1 = p_t1.tile([128, M, TB], DT, tag="t1", name="t1")
            nc.vector.tensor_mul(t1, psb_sb, kp[r])
            ct = p_ct.tile([128, M, TB], DT, tag="ct", name="ct")
            nc.vector.tensor_mul(ct, t1, kp[r])
            tmp = p_tmp.tile([128, 2, TB], DT, tag="tmp", name="tmp")
            nc.vector.tensor_add(tmp, ct[:, 0:2, :], ct[:, 2:4, :])
            nc.vector.tensor_add(yb[:, r, :], tmp[:, 0, :], tmp[:, 1, :])

        def _store_out(blk, tt, oc, ps):
            t0 = blk * TB
            if with_bias:
                nc.tensor.matmul(
                    ps,
                    ones_sb,
                    bo_sb[:, oc * 512:(oc + 1) * 512],
                    start=False,
                    stop=True,
                )
            o_sb = p_out.tile([128, 512], F32, tag="outs", name="osb")
            nc.scalar.copy(out=o_sb, in_=ps)
            nc.sync.dma_start(
                out=out.ap()[t0 + tt * 128:t0 + (tt + 1) * 128,
                             oc * 512:(oc + 1) * 512],
                in_=o_sb,
            )

        def emit_outproj_tt0_r(blk, r):
            # tt=0 output groups accumulate one r-chunk at a time, spread
            # through the attend stream (lagged so yb[:, r] is ready)
            yb = st[blk]["yb"]
            if r == 0:
                st[blk]["po"] = [po.tile([128, 512], F32, tag="po", name="po")
                                 for _ in range(2)]
            for oc in range(2):
                nc.tensor.matmul(
                    st[blk]["po"][oc],
                    yb[:, r, 0:128],
                    wo_sb[:, r, oc * 512:(oc + 1) * 512],
                    start=(r == 0),
                    stop=(r == NJ - 1 and not with_bias),
                )

        def emit_outproj_finish(blk):
            yb = st[blk]["yb"]
            for oc in range(2):
                _store_out(blk, 0, oc, st[blk]["po"][oc])
            for tt in range(1, NT):
                for oc in range(2):
                    ps = po.tile([128, 512], F32, tag="po", name="po")
                    for r in range(NJ):
                        nc.tensor.matmul(
                            ps,
                            yb[:, r, tt * 128:(tt + 1) * 128],
                            wo_sb[:, r, oc * 512:(oc + 1) * 512],
                            start=(r == 0),
                            stop=(r == NJ - 1 and not with_bias),
                        )
                    _store_out(blk, tt, oc, ps)

        # ---- software-pipelined emission ----
        emit_load(0)
        emit_ind_loads()
        for blk in range(nblk + 1):
            cur = blk if blk < nblk else None
            prv = blk - 1 if blk > 0 else None
            if cur is not None and blk + 1 < nblk and blk > 0:
                emit_load(blk + 1)
            if prv is not None:
                emit_prods(prv)
            if cur is not None:
                emit_qproj(cur)
            if prv is not None:
                emit_logits(prv)
            if cur is not None:
                for r in range(NJ):
                    emit_kproj_pair(cur, 0, r)
            if prv is not None:
                emit_softmax(prv)
            # interleave attends of prv (1 per m-pair k-proj r-group) and
            # out-proj tt0 r-chunks (lagged one attend) so the PE always has
            # dense work while the attend DVE/ACT chains drain
            if cur is not None:
                ai = 0
                for r in range(NJ):
                    emit_kproj_pair(cur, 1, r)
                    if prv is not None:
                        emit_attend(prv, ai)
                        if ai > 0:
                            emit_outproj_tt0_r(prv, ai - 1)
                        ai += 1
            elif prv is not None:
                for r in range(NJ):
                    emit_attend(prv, r)
                    if r > 0:
                        emit_outproj_tt0_r(prv, r - 1)
            if prv is not None:
                emit_outproj_tt0_r(prv, NJ - 1)
                emit_outproj_finish(prv)
                st[prv].clear()
            # block-1 loads emitted after block-0's work so the startup DMAs
            # (weights + block-0 inputs) get exclusive HBM bandwidth
            if blk == 0 and nblk > 1:
                emit_load(1)
    nc.compile()
    return nc


def _make_indicators():
    # row index = 16*mp + h (h = 2r + channel-half), free carries mi (m=2mp+mi)
    p = np.arange(128)[:, None]
    indl = np.zeros((128, 16 * 32), dtype=NPDT)
    indb = np.zeros((32, 16 * 128), dtype=NPDT)
    for r in range(NJ):
        for mp in range(2):
            k = 2 * r + mp
            j = np.arange(32)[None, :]
            indl[:, k * 32:(k + 1) * 32] = (
                j == 16 * mp + 2 * r + (p >= 64)
            ).astype(NPDT)
            c = np.arange(128)[None, :]
            indb[:, k * 128:(k + 1) * 128] = (
                np.arange(32)[:, None] == 16 * mp + 2 * r + (c >= 64)
            ).astype(NPDT)
    ps = np.arange(32)[:, None]
    js = np.arange(32)[None, :]
    inds = (ps % 16 == js % 16).astype(NPDT)
    return indl, inds, indb


def _host_prep(query, key, Wq, Wk, Wo, bq, bk, bo):
    qT = np.ascontiguousarray(
        np.asarray(query).reshape(T_TOTAL, C).T).astype(NPDT)
    kT = np.asarray(key).reshape(T_TOTAL, M, C).transpose(1, 2, 0)  # [M,C,T]

    wqT = np.ascontiguousarray(np.asarray(Wq).T).astype(NPDT)
    wkT = np.ascontiguousarray(np.asarray(Wk).T).astype(NPDT)
    woT = np.ascontiguousarray(np.asarray(Wo).T).astype(NPDT)

    indl, inds, indb = _make_indicators()

    with_bias = bool(np.any(bq) or np.any(bk) or np.any(bo))
    common = {"wqT": wqT, "wkT": wkT, "woT": woT,
              "indl": indl, "inds": inds, "indb": indb}
    if with_bias:
        common |= {
            "bqp": np.ascontiguousarray(
                np.asarray(bq).reshape(NJ, 128).T).astype(np.float32),
            "bkp": np.ascontiguousarray(
                np.asarray(bk).reshape(NJ, 128).T).astype(np.float32),
            "bo": np.asarray(bo).reshape(1, C).astype(NPDT),
        }
    in_maps = []
    nblk = T_CORE // TB
    for i in range(NCORES):
        sl = slice(i * T_CORE, (i + 1) * T_CORE)
        # blocked m-pair layout [mp, blk, C, mi, TB]
        kc = kT[:, :, sl].reshape(2, 2, C, nblk, TB).transpose(0, 3, 2, 1, 4)
        in_maps.append(
            {
                "qT": np.ascontiguousarray(qT[:, sl]),
                "kT": np.ascontiguousarray(kc).astype(NPDT),
                **common,
            }
        )
    return in_maps, with_bias


_NC_CACHE = {}
_LAST_RESULT = None


def kernel(query, key, gate, Wq, bq, Wk, bk, Wv, bv, Wg, bg, Wo, bo):
    in_maps, with_bias = _host_prep(query, key, Wq, Wk, Wo, bq, bk, bo)
    key_ = (T_CORE, with_bias)
    if key_ not in _NC_CACHE:
        _NC_CACHE[key_] = build_nc(T_CORE, with_bias)
    nc = _NC_CACHE[key_]
    res = run_bass_kernel_spmd(nc, in_maps, list(range(NCORES)))
    global _LAST_RESULT
    _LAST_RESULT = res
    out = np.concatenate([res.results[i]["out"] for i in range(NCORES)], axis=0)
    return out.reshape(B, N, C)
